# revision 1
# baseline (speedup 1.0000x reference)
"""BitLinear-1.58 (absmean ternary quantized linear) Trainium2 kernel.

Full-input contract: kernel(x[4,4096,4096] f32, weight[4096,4096] f32)
-> [4,4096,4096] f32, computing x @ Wq.T with
Wq = sign(W) * clip(round(|W|/gamma), 0, 1), gamma = mean(|W|) + 1e-6.

Sharding: data-parallel over tokens. Each of the 8 cores processes 2048
of the 16384 (b, s) rows with the full weight replicated; no collectives.

The scalar quantization threshold thr = gamma/2 is computed on the host
with the exact same jax-on-CPU op the reference uses (jnp.mean of |W|),
so the ternary decision boundary is bit-identical to the reference's;
knife-edge weights sit within one ulp of the threshold and would
otherwise flip. All O(N^3) compute and the full elementwise
quantization run on device.

Per-core pipeline (software-pipelined; emission order is per-engine
program order):
  - x loaded once, cast f32->f16 on ACT, transposed k-major on the PE
    (fp16 transpose-mode matmuls through an identity, PSUM->SBUF
    copyback) into a fully resident xT; no DRAM round-trip.
  - W quantized on DVE (q = (W > thr) - (W < -thr) in fp16), staged to
    DRAM, reloaded k-major per 256-column n-block with one XBAR
    transpose-DMA, double-buffered behind the previous block's matmuls.
  - Matmul: out[m128, n256] += xT[k128, m128].T @ WqT[k128, n256]
    accumulated over 32 k-tiles in PSUM (fp32), evicted via DVE copy.
"""

from contextlib import ExitStack

import numpy as np

import concourse.bass as bass
import concourse.mybir as mybir
import concourse.tile as tile
from concourse import bacc
from concourse.bass_utils import run_bass_kernel_spmd
from concourse.masks import make_identity

FP32 = mybir.dt.float32
FP16 = mybir.dt.float16

P = 128
EPS = 1e-6
N_CORES = 8

# Full-problem dims (hardcoded per harness contract)
B, S, D_IN, D_OUT = 4, 4096, 4096, 4096
M_FULL = B * S
M_LOC = M_FULL // N_CORES


def _bitlinear_body(ctx, tc, out_ap, x_ap, w_ap, thr_ap, nthr_ap,
                    M_loc, D_in, D_out, N_blk):
    nc = tc.nc
    KB = D_in // P              # k-tiles of 128
    NT = D_out // P             # weight row-tiles of 128
    KC = min(D_in, 1024)        # free-dim chunk for prep DMAs
    NCH = D_in // KC            # chunks per row-tile
    HK = min(D_in, 2048)        # x columns cast+transposed per group
    NHK = D_in // HK            # column groups per x row-tile
    KBH = HK // P               # k-tiles per column group
    MT = M_loc // P             # m-tiles
    MB = min(M_loc, 512)        # rows per xT sub-tile
    NMB = M_loc // MB           # xT sub-tiles
    MTB = MB // P               # m-tiles per xT sub-tile
    NB = D_out // N_blk         # n-blocks
    NBT = N_blk // P            # weight row-tiles per n-block

    dram = ctx.enter_context(tc.tile_pool(name="dram", bufs=1, space="DRAM"))
    wq16 = dram.tile([D_out, D_in], FP16)

    stats = ctx.enter_context(tc.tile_pool(name="stats", bufs=1, side="left"))
    thr_b = stats.tile([P, 1], FP32)
    nc.sync.dma_start(thr_b[:], thr_ap)
    nthr_b = stats.tile([P, 1], FP32)
    nc.sync.dma_start(nthr_b[:], nthr_ap)
    ident = stats.tile([P, P], FP16)
    make_identity(nc, ident[:])

    # prep pools cycle small tiles on the left; long-lived matmul-phase
    # tiles allocate from the right end so prep churn can't fragment them
    ld = ctx.enter_context(tc.tile_pool(name="ld", bufs=2, side="left"))
    q16 = ctx.enter_context(tc.tile_pool(name="q16", bufs=3, side="left"))
    xq16 = ctx.enter_context(tc.tile_pool(name="xq16", bufs=2, side="left"))
    cmp = ctx.enter_context(tc.tile_pool(name="cmp", bufs=1, side="left"))
    co = ctx.enter_context(tc.tile_pool(name="co", bufs=2, side="left"))
    xT = ctx.enter_context(tc.tile_pool(name="xT", bufs=NMB, side="right"))
    wqt = ctx.enter_context(tc.tile_pool(name="wqt", bufs=2, side="right"))
    ps = ctx.enter_context(tc.tile_pool(name="ps", bufs=4, space="PSUM"))
    tp = ctx.enter_context(tc.tile_pool(name="tp", bufs=4, space="PSUM"))

    def quant_chunk(nt, h):
        wt = ld.tile([P, KC], FP32, tag="ld")
        nc.sync.dma_start(wt[:], w_ap[nt * P:(nt + 1) * P, h * KC:(h + 1) * KC])
        a = cmp.tile([P, KC], FP16, tag="a")
        nc.vector.tensor_scalar(
            a[:], wt[:], thr_b[:], None, mybir.AluOpType.is_gt)
        bneg = cmp.tile([P, KC], FP16, tag="b")
        nc.vector.tensor_scalar(
            bneg[:], wt[:], nthr_b[:], None, mybir.AluOpType.is_lt)
        qt = q16.tile([P, KC], FP16, tag="q16")
        nc.vector.tensor_tensor(qt[:], a[:], bneg[:], mybir.AluOpType.subtract)
        nc.sync.dma_start(wq16[nt * P:(nt + 1) * P, h * KC:(h + 1) * KC], qt[:])

    def quant_w(nt):
        for h in range(NCH):
            quant_chunk(nt, h)

    xTts = [None] * NMB

    def xt_tile(mb):
        if xTts[mb] is None:
            xTts[mb] = xT.tile([P, KB, MB], FP16, tag="xT", name=f"xTt{mb}")
        return xTts[mb]

    def load_x(mt):
        # load+cast one x row-tile, transpose k-major on the PE into xT
        t = xt_tile(mt // MTB)
        mc = (mt % MTB) * P
        for g in range(NHK):
            xq = xq16.tile([P, HK], FP16, tag="xq")
            for h in range(HK // KC):
                c = g * HK + h * KC
                xt_ = ld.tile([P, KC], FP32, tag="ld")
                nc.sync.dma_start(xt_[:], x_ap[mt * P:(mt + 1) * P, c:c + KC])
                nc.scalar.activation(
                    xq[:, h * KC:(h + 1) * KC], xt_[:],
                    mybir.ActivationFunctionType.Copy)
            for j in range(KBH):
                pt = tp.tile([P, P], FP16)
                nc.tensor.transpose(pt[:], xq[:, j * P:(j + 1) * P], ident[:])
                # alternate copyback engine: ACT also runs the casts
                eng = nc.vector if j % 2 == 0 else nc.scalar
                if eng is nc.vector:
                    eng.tensor_copy(
                        out=t[:, g * KBH + j, mc:mc + P], in_=pt[:])
                else:
                    nc.scalar.activation(
                        t[:, g * KBH + j, mc:mc + P], pt[:],
                        mybir.ActivationFunctionType.Copy)

    def matmuls(nb, wq_t, mts):
        for mt in mts:
            xTt = xTts[mt // MTB]
            mc = (mt % MTB) * P
            pst = ps.tile([P, N_blk], FP32)
            for kb in range(KB):
                nc.tensor.matmul(
                    pst[:],
                    xTt[:, kb, mc:mc + P],
                    wq_t[:, kb, :],
                    start=(kb == 0),
                    stop=(kb == KB - 1),
                )
            cot = co.tile([P, N_blk], FP32)
            nc.vector.tensor_copy(out=cot[:], in_=pst[:])
            nc.sync.dma_start(
                out_ap[mt * P:(mt + 1) * P, nb * N_blk:(nb + 1) * N_blk],
                cot[:],
            )

    def wqt_load(nb):
        wq_t = wqt.tile([P, KB, N_blk], FP16, tag="wq_t")
        nc.sync.dma_start_transpose(
            wq_t[:], wq16[nb * N_blk:(nb + 1) * N_blk, :])
        return wq_t

    # startup: quantize n-blocks 0..1 and interleave x ingestion with
    # their matmuls one 512-row group at a time, so the PE has enough
    # work to cover the ingest stream
    quant_done = set()
    second = 1 < NB
    # background quantize chunks for n-blocks 1..2, pumped between x
    # tile loads so neither the PE's x feed nor the weight feed starves
    bg = [(nt, h)
          for nt in range(NBT, min(3 * NBT, NT))
          for h in range(NCH)] if second else []
    bgpos = [0]

    def pump(n):
        while n > 0 and bgpos[0] < len(bg):
            nt, h = bg[bgpos[0]]
            quant_chunk(nt, h)
            bgpos[0] += 1
            n -= 1

    nb1_chunks = NBT * NCH if second else 0

    # first x rows ahead of the weight stream: PE transposes start early
    load_x(0)
    for nt in range(NBT):
        quant_w(nt)
    wq_t0 = wqt_load(0)
    matmuls(0, wq_t0, [0])
    for mt in range(1, MTB):
        load_x(mt)
        matmuls(0, wq_t0, [mt])

    wq_t1 = None
    for mb in range(1, NMB):
        for mt in range(mb * MTB, (mb + 1) * MTB):
            pump(2)
            load_x(mt)
            matmuls(0, wq_t0, [mt])
            if wq_t1 is not None:
                matmuls(1, wq_t1, [mt])
        if second and wq_t1 is None:
            pump(nb1_chunks - bgpos[0])  # ensure n-block 1 fully staged
            wq_t1 = wqt_load(1)
            matmuls(1, wq_t1, range((mb + 1) * MTB))
    if second and wq_t1 is None:
        pump(nb1_chunks - bgpos[0])
        wq_t1 = wqt_load(1)
        matmuls(1, wq_t1, range(MT))
    pump(len(bg))  # drain remaining background chunks (n-block 2)
    if len(bg) > nb1_chunks:
        quant_done.add(2)

    for nb in range(2, NB):
        if nb not in quant_done:
            for nt in range(nb * NBT, (nb + 1) * NBT):
                quant_w(nt)
        wq_t = wqt_load(nb)
        matmuls(nb, wq_t, range(MT))


def build_nc(M_loc=M_LOC, D_in=D_IN, D_out=D_OUT, N_blk=256):
    nc = bacc.Bacc("TRN2", target_bir_lowering=False, debug=False,
                   num_devices=N_CORES)
    x = nc.dram_tensor("x", [M_loc, D_in], FP32, kind="ExternalInput").ap()
    w = nc.dram_tensor("w", [D_out, D_in], FP32, kind="ExternalInput").ap()
    thr = nc.dram_tensor("thr", [P, 1], FP32, kind="ExternalInput").ap()
    nthr = nc.dram_tensor("nthr", [P, 1], FP32, kind="ExternalInput").ap()
    out = nc.dram_tensor("out", [M_loc, D_out], FP32, kind="ExternalOutput").ap()
    with tile.TileContext(nc) as tc:
        with ExitStack() as ctx:
            _bitlinear_body(ctx, tc, out, x, w, thr, nthr,
                            M_loc, D_in, D_out, N_blk)
    nc.compile()
    return nc


_NC = None


def _get_nc():
    global _NC
    if _NC is None:
        _NC = build_nc()
    return _NC


def _host_threshold(weight: np.ndarray) -> np.float32:
    """gamma/2 with gamma bit-identical to the reference's jax-on-CPU mean."""
    import jax
    import jax.numpy as jnp

    cpu = jax.devices("cpu")[0]
    with jax.default_device(cpu):
        gamma = jnp.mean(jnp.abs(jnp.asarray(weight, dtype=jnp.float32)))
    gamma = np.float32(gamma) + np.float32(EPS)
    return np.float32(gamma * np.float32(0.5))


def kernel(x: np.ndarray, weight: np.ndarray, **_ignored) -> np.ndarray:
    assert x.shape == (B, S, D_IN) and weight.shape == (D_OUT, D_IN)
    xf = np.ascontiguousarray(x.reshape(M_FULL, D_IN).astype(np.float32, copy=False))
    w = np.ascontiguousarray(weight.astype(np.float32, copy=False))
    thr = _host_threshold(w)
    thr_arr = np.full((P, 1), thr, dtype=np.float32)
    nthr_arr = -thr_arr
    nc = _get_nc()
    in_maps = [
        {"x": np.ascontiguousarray(xf[i * M_LOC:(i + 1) * M_LOC]), "w": w,
         "thr": thr_arr, "nthr": nthr_arr}
        for i in range(N_CORES)
    ]
    res = run_bass_kernel_spmd(nc, in_maps, core_ids=list(range(N_CORES)))
    outs = [res.results[i]["out"] for i in range(N_CORES)]
    full = np.concatenate(outs, axis=0)
    if not np.isfinite(full).all():
        # cold-start transient guard: retry once
        res = run_bass_kernel_spmd(nc, in_maps, core_ids=list(range(N_CORES)))
        outs = [res.results[i]["out"] for i in range(N_CORES)]
        full = np.concatenate(outs, axis=0)
    return full.reshape(B, S, D_OUT).astype(np.float32, copy=False)


if __name__ == "__main__":
    # quick smoke on small shapes via CoreSim
    from concourse.bass_interp import CoreSim

    M_loc, D_in, D_out = 256, 512, 512
    nc = build_nc(M_loc=M_loc, D_in=D_in, D_out=D_out, N_blk=256)
    rng = np.random.default_rng(0)
    xs = rng.standard_normal((M_loc, D_in), dtype=np.float32)
    ws = rng.standard_normal((D_out, D_in), dtype=np.float32)
    gamma = np.abs(ws).mean(dtype=np.float32) + np.float32(EPS)
    thr = np.float32(gamma * np.float32(0.5))
    sim = CoreSim(nc, require_finite=True, require_nnan=True)
    sim.tensor("x")[:] = xs
    sim.tensor("w")[:] = ws
    sim.tensor("thr")[:] = np.full((P, 1), thr, np.float32)
    sim.tensor("nthr")[:] = np.full((P, 1), -thr, np.float32)
    sim.simulate(check_with_hw=False)
    got = np.array(sim.tensor("out"))

    wq = np.sign(ws) * np.clip(np.round(np.abs(ws / gamma)), None, 1.0)
    exp = xs @ wq.T.astype(np.float32)
    err = np.abs(got - exp).max() / np.abs(exp).max()
    print("sim rel err:", err)



# revision 5
# speedup vs baseline: 1.7147x; 1.7147x over previous
"""BitLinear-1.58 (absmean ternary quantized linear) Trainium2 kernel.

Full-input contract: kernel(x[4,4096,4096] f32, weight[4096,4096] f32)
-> [4,4096,4096] f32, computing x @ Wq.T with
Wq = sign(W) * clip(round(|W|/gamma), 0, 1), gamma = mean(|W|) + 1e-6.

Sharding: data-parallel over tokens. Each of the 8 cores processes 2048
of the 16384 (b, s) rows with the full weight replicated; no collectives.

The scalar quantization threshold thr = gamma/2 is computed on the host
with the exact same jax-on-CPU op the reference uses (jnp.mean of |W|),
so the ternary decision boundary is bit-identical to the reference's.
All O(N^3) compute and the full elementwise quantization run on device.

Math strategy: fp8 DoubleRow matmuls at 2x PE throughput. x is split
exactly into x ~= hi + lo with hi = e4m3(x), lo = e4m3(x - hi)
(residual <= 2^-8 relative, final rel err ~1e-3), and Wq in {-1,0,+1}
is exact in e4m3. Each DoubleRow matmul contracts TWO 128-deep k-slabs
(0.5 cycles per output row); a hi pass and a lo pass over k-slab pairs
accumulate into the same PSUM group, so the full f32-accuracy product
costs half the fp16 PE time.

Per-core pipeline (no DRAM staging, no fp16 intermediates):
  - x loaded f32, transposed k-major on the PE (f32 transpose mode),
    then split from PSUM: ACT casts psum->xhiT (e4m3), DVE computes
    psum - hi -> xloT (e4m3).
  - W loaded f32 per 128-row tile, quantized on DVE in two passes
    (b = (w < -thr), then q = (w > thr) - b -> e4m3, decisions in f32,
    bit-identical to the reference), PE-transposed k-major per 128x128
    tile, copied back PSUM->SBUF by ACT into 512-column n-blocks.
  - Matmul per (n-block, m-tile): 64 DoubleRow fp8 matmuls (2 halves x
    (hi+lo) x 16 k-slab-pairs) accumulate [128, 512] f32 in PSUM,
    evicted by ACT, DMA'd out. W prep for block nb+1 is pumped between
    m-tiles of block nb; x ingest overlaps block 0's matmuls.
"""

from contextlib import ExitStack

import numpy as np

import concourse.bass as bass
import concourse.mybir as mybir
import concourse.tile as tile
from concourse import bacc
from concourse.bass_utils import run_bass_kernel_spmd
from concourse.masks import make_identity

FP32 = mybir.dt.float32
FP16 = mybir.dt.float16
FP8 = mybir.dt.float8e4

P = 128
EPS = 1e-6
N_CORES = 8

# Full-problem dims (hardcoded per harness contract)
B, S, D_IN, D_OUT = 4, 4096, 4096, 4096
M_FULL = B * S
M_LOC = M_FULL // N_CORES

Copy = mybir.ActivationFunctionType.Copy
DoubleRow = mybir.MatmulPerfMode.DoubleRow


def _bitlinear_body(ctx, tc, out_ap, x_ap, w_ap, thr_ap, nthr_ap,
                    M_loc, D_in, D_out):
    nc = tc.nc
    KC = min(1024, D_in)        # f32 chunk (free dim) for DMA + quantize
    G = min(8, D_in // P)       # 128x128 transposes per PSUM group
    KB = D_in // P              # k-slabs of 128
    KP = KB // 2                # k-slab pairs per DoubleRow pass
    MT = M_loc // P             # m-tiles
    NBW = min(512, D_out)       # n-block width (columns of Wq.T)
    NB = D_out // NBW           # n-blocks
    RT = NBW // P               # W row-tiles per n-block
    OC = min(256, NBW)          # psum out columns per matmul
    NH = NBW // OC              # out column chunks per n-block

    stats = ctx.enter_context(tc.tile_pool(name="stats", bufs=1, side="left"))
    thr_b = stats.tile([P, 1], FP32)
    nc.sync.dma_start(thr_b[:], thr_ap)
    nthr_b = stats.tile([P, 1], FP32)
    nc.sync.dma_start(nthr_b[:], nthr_ap)
    ident32 = stats.tile([P, P], FP32)
    make_identity(nc, ident32[:])
    ident16 = stats.tile([P, P], FP16)
    make_identity(nc, ident16[:])

    # streaming pools on the left; long-lived k-major tensors on the right
    xld = ctx.enter_context(tc.tile_pool(name="xld", bufs=2, side="left"))
    wld = ctx.enter_context(tc.tile_pool(name="wld", bufs=2, side="left"))
    bq = ctx.enter_context(tc.tile_pool(name="bq", bufs=2, side="left"))
    qrow = ctx.enter_context(tc.tile_pool(name="qrow", bufs=2, side="left"))
    co = ctx.enter_context(tc.tile_pool(name="co", bufs=3, side="left"))
    xT = ctx.enter_context(tc.tile_pool(name="xT", bufs=1, side="right"))
    wqt = ctx.enter_context(tc.tile_pool(name="wqt", bufs=2, side="right"))
    ps = ctx.enter_context(tc.tile_pool(name="ps", bufs=2, space="PSUM"))
    tp = ctx.enter_context(tc.tile_pool(name="tp", bufs=2, space="PSUM"))
    tw = ctx.enter_context(tc.tile_pool(name="tw", bufs=2, space="PSUM"))

    xhiT = xT.tile([P, KB, M_loc], FP8, name="xhiT")
    xloT = xT.tile([P, KB, M_loc], FP8, name="xloT")

    wq_bufs = {}

    def wq_rowtile(nb, rt):
        """Quantize + transpose one 128-row tile of W into block nb."""
        if rt == 0:
            wq_bufs[nb] = wqt.tile([P, KB, NBW], FP8, tag="wqt",
                                   name=f"wqt{nb % 2}")
        wq_t = wq_bufs[nb]
        qrow_t = qrow.tile([P, D_in], FP16, tag="qrow")
        r = nb * RT + rt
        for h in range(D_in // KC):
            wt = wld.tile([P, KC], FP32, tag="wld")
            nc.sync.dma_start(wt[:], w_ap[r * P:(r + 1) * P, h * KC:(h + 1) * KC])
            b = bq.tile([P, KC], FP16, tag="bq")
            nc.vector.tensor_scalar(
                b[:], wt[:], nthr_b[:], None, mybir.AluOpType.is_lt)
            nc.vector.scalar_tensor_tensor(
                qrow_t[:, h * KC:(h + 1) * KC], wt[:], thr_b[:], b[:],
                mybir.AluOpType.is_gt, mybir.AluOpType.subtract)
        for g in range(KB // G):
            twt = tw.tile([P, G, P], FP16, tag="tw")
            for j in range(G):
                k = g * G + j
                nc.tensor.transpose(
                    twt[:, j, :], qrow_t[:, k * P:(k + 1) * P], ident16[:])
            nc.scalar.activation(
                wq_t[:, g * G:(g + 1) * G, rt * P:(rt + 1) * P], twt[:], Copy)
        return wq_t

    def ingest(mt):
        """Load one 128-row x tile, transpose k-major, split hi/lo fp8."""
        for q in range(D_in // KC):
            xt = xld.tile([P, KC], FP32, tag="xld")
            nc.sync.dma_start(xt[:], x_ap[mt * P:(mt + 1) * P, q * KC:(q + 1) * KC])
            tpt = tp.tile([P, G, P], FP32, tag="tp")
            for j in range(G):
                nc.tensor.transpose(
                    tpt[:, j, :], xt[:, j * P:(j + 1) * P], ident32[:])
            kb0 = q * G
            hi = xhiT[:, kb0:kb0 + G, mt * P:(mt + 1) * P]
            nc.scalar.activation(hi, tpt[:], Copy)
            nc.vector.scalar_tensor_tensor(
                xloT[:, kb0:kb0 + G, mt * P:(mt + 1) * P], tpt[:], 1.0, hi,
                mybir.AluOpType.mult, mybir.AluOpType.subtract)

    def matmuls(nb, mt, wq_t):
        pst = ps.tile([P, NBW], FP32, tag="ps")
        for h in range(NH):
            o = pst[:, h * OC:(h + 1) * OC]
            for si, src in enumerate((xhiT, xloT)):
                for kp in range(KP):
                    nc.tensor.matmul(
                        o,
                        src[:, 2 * kp:2 * kp + 2, mt * P:(mt + 1) * P],
                        wq_t[:, 2 * kp:2 * kp + 2, h * OC:(h + 1) * OC],
                        start=(si == 0 and kp == 0),
                        stop=(si == 1 and kp == KP - 1),
                        perf_mode=DoubleRow,
                    )
        cot = co.tile([P, NBW], FP32, tag="co")
        nc.scalar.activation(cot[:], pst[:], Copy)
        nc.sync.dma_start(
            out_ap[mt * P:(mt + 1) * P, nb * NBW:(nb + 1) * NBW], cot[:])

    # ---- schedule ----
    # Block 0 W-prep up front, then x ingest interleaved with block-0
    # matmuls m-tile by m-tile; block nb+1's row-tiles are pumped at
    # evenly spaced m-tile slots during block nb's matmul sweep.
    def pump_slots():
        # RT pump positions inside an MT-long sweep, starting at 1
        step = max(1, (MT - 1) // RT)
        return {1 + i * step: i for i in range(RT)}

    for rt in range(RT):
        wq_rowtile(0, rt)
    wq0 = wq_bufs[0]
    slots = pump_slots() if NB > 1 else {}
    ingest(0)
    matmuls(0, 0, wq0)
    done_next = 0
    for mt in range(1, MT):
        ingest(mt)
        matmuls(0, mt, wq0)
        if mt in slots and NB > 1:
            wq_rowtile(1, slots[mt])
            done_next += 1
    for nb in range(1, NB):
        while done_next < RT:  # finish any unpumped row-tiles
            wq_rowtile(nb, done_next)
            done_next += 1
        wq_t = wq_bufs[nb]
        done_next = 0
        slots = pump_slots() if nb + 1 < NB else {}
        for mt in range(MT):
            matmuls(nb, mt, wq_t)
            if mt in slots:
                wq_rowtile(nb + 1, slots[mt])
                done_next += 1


def build_nc(M_loc=M_LOC, D_in=D_IN, D_out=D_OUT):
    nc = bacc.Bacc("TRN2", target_bir_lowering=False, debug=False,
                   num_devices=N_CORES)
    x = nc.dram_tensor("x", [M_loc, D_in], FP32, kind="ExternalInput").ap()
    w = nc.dram_tensor("w", [D_out, D_in], FP32, kind="ExternalInput").ap()
    thr = nc.dram_tensor("thr", [P, 1], FP32, kind="ExternalInput").ap()
    nthr = nc.dram_tensor("nthr", [P, 1], FP32, kind="ExternalInput").ap()
    out = nc.dram_tensor("out", [M_loc, D_out], FP32, kind="ExternalOutput").ap()
    with tile.TileContext(nc) as tc:
        with ExitStack() as ctx:
            _bitlinear_body(ctx, tc, out, x, w, thr, nthr,
                            M_loc, D_in, D_out)
    nc.compile()
    return nc


_NC = None


def _get_nc():
    global _NC
    if _NC is None:
        _NC = build_nc()
    return _NC


def _host_threshold(weight: np.ndarray) -> np.float32:
    """gamma/2 with gamma bit-identical to the reference's jax-on-CPU mean."""
    import jax
    import jax.numpy as jnp

    cpu = jax.devices("cpu")[0]
    with jax.default_device(cpu):
        gamma = jnp.mean(jnp.abs(jnp.asarray(weight, dtype=jnp.float32)))
    gamma = np.float32(gamma) + np.float32(EPS)
    return np.float32(gamma * np.float32(0.5))


def kernel(x: np.ndarray, weight: np.ndarray, **_ignored) -> np.ndarray:
    assert x.shape == (B, S, D_IN) and weight.shape == (D_OUT, D_IN)
    xf = np.ascontiguousarray(x.reshape(M_FULL, D_IN).astype(np.float32, copy=False))
    w = np.ascontiguousarray(weight.astype(np.float32, copy=False))
    thr = _host_threshold(w)
    thr_arr = np.full((P, 1), thr, dtype=np.float32)
    nthr_arr = -thr_arr
    nc = _get_nc()
    in_maps = [
        {"x": np.ascontiguousarray(xf[i * M_LOC:(i + 1) * M_LOC]), "w": w,
         "thr": thr_arr, "nthr": nthr_arr}
        for i in range(N_CORES)
    ]
    res = run_bass_kernel_spmd(nc, in_maps, core_ids=list(range(N_CORES)))
    outs = [res.results[i]["out"] for i in range(N_CORES)]
    full = np.concatenate(outs, axis=0)
    if not np.isfinite(full).all():
        # cold-start transient guard: retry once
        res = run_bass_kernel_spmd(nc, in_maps, core_ids=list(range(N_CORES)))
        outs = [res.results[i]["out"] for i in range(N_CORES)]
        full = np.concatenate(outs, axis=0)
    return full.reshape(B, S, D_OUT).astype(np.float32, copy=False)


if __name__ == "__main__":
    # quick smoke on small shapes via CoreSim
    from concourse.bass_interp import CoreSim

    M_loc, D_in, D_out = 256, 512, 512
    nc = build_nc(M_loc=M_loc, D_in=D_in, D_out=D_out)
    rng = np.random.default_rng(0)
    xs = rng.standard_normal((M_loc, D_in), dtype=np.float32)
    ws = rng.standard_normal((D_out, D_in), dtype=np.float32)
    gamma = np.abs(ws).mean(dtype=np.float32) + np.float32(EPS)
    thr = np.float32(gamma * np.float32(0.5))
    sim = CoreSim(nc, require_finite=True, require_nnan=True)
    sim.tensor("x")[:] = xs
    sim.tensor("w")[:] = ws
    sim.tensor("thr")[:] = np.full((P, 1), thr, np.float32)
    sim.tensor("nthr")[:] = np.full((P, 1), -thr, np.float32)
    sim.simulate(check_with_hw=False)
    got = np.array(sim.tensor("out"))

    wq = np.sign(ws) * np.clip(np.round(np.abs(ws / gamma)), None, 1.0)
    exp = xs @ wq.T.astype(np.float32)
    err = np.abs(got - exp).max() / np.abs(exp).max()
    print("sim rel err:", err)
